# revision 1
# baseline (speedup 1.0000x reference)
"""Trainium2 Bass kernel for nn_C3AH (C3-style hypergraph attention block).

Contract: kernel(**inputs) takes the FULL unsharded inputs (numpy f32) and
returns the FULL output [16, 256, 64, 64] f32.  Internally: data-parallel over
batch across 8 NeuronCores (2 batches per core), weights replicated, all
heavy matmuls in bf16 with f32 PSUM accumulation.

Key algebraic transforms (validated in numpy against the reference):
  - BN folded into 1x1-conv weights; conv+BN+SiLU = one matmul + ACT Silu.
  - mean over heads of per-head logits == full-C dot / NH  -> logits =
    (protos @ pre_w) @ tokens / 64;  pre_b drops out (softmax shift-invar).
  - ctx_b folded into proto.
  - node linear fused through the rank-E hyperedge bottleneck:
    node_w @ Xn^T = (node_w @ He_out^T) @ A  (E=8 contraction).
  - softmax normalization (1/Z) folded into the two A-applications
    (He aggregation copy, W_He lhsT copy); Z comes free from ACT accum_out.
"""
import sys
import functools

sys.path.insert(0, "/opt/trn_rl_repo")

import numpy as np
import ml_dtypes

import concourse.bass as bass
import concourse.tile as tile
from concourse import bacc, mybir
from concourse.bass_utils import run_bass_kernel_spmd

BF16 = ml_dtypes.bfloat16
FP32 = mybir.dt.float32
BF = mybir.dt.bfloat16
AF = mybir.ActivationFunctionType
AX = mybir.AxisListType

B, C1, H, W = 16, 256, 64, 64
N = H * W            # 4096
CH, C2, E = 256, 256, 8
NCORES = 8
BLOC = B // NCORES   # 2 batches per core
EPS = 1e-5
LSCALE = 1.0 / 64.0  # 1/(NH*sqrt(HD))

NCH = 2048           # free-dim chunk for big PSUM tiles / ACT calls
NSUB = 512           # matmul moving-operand max


def emit_kernel(nc):
    # ---------------- DRAM I/O ----------------
    x_d = nc.dram_tensor("x", [BLOC, C1, N], BF, kind="ExternalInput")
    w1t_d = nc.dram_tensor("w1t", [C1, CH], BF, kind="ExternalInput")
    w2t_d = nc.dram_tensor("w2t", [C1, CH], BF, kind="ExternalInput")
    w3t_d = nc.dram_tensor("w3t", [2 * CH, C2], BF, kind="ExternalInput")
    prew_d = nc.dram_tensor("prew", [CH, CH], BF, kind="ExternalInput")
    protot_d = nc.dram_tensor("protot", [CH, E], BF, kind="ExternalInput")
    ctxwt_d = nc.dram_tensor("ctxwt", [2 * CH, E * CH], BF, kind="ExternalInput")
    edgewt_d = nc.dram_tensor("edgewt", [CH, CH], BF, kind="ExternalInput")
    nodewt_d = nc.dram_tensor("nodewt", [CH, CH], BF, kind="ExternalInput")
    b1_d = nc.dram_tensor("b1", [CH], FP32, kind="ExternalInput")
    b2_d = nc.dram_tensor("b2", [CH], FP32, kind="ExternalInput")
    b3_d = nc.dram_tensor("b3", [C2], FP32, kind="ExternalInput")
    eb_d = nc.dram_tensor("eb", [CH], FP32, kind="ExternalInput")
    nb_d = nc.dram_tensor("nb", [CH], FP32, kind="ExternalInput")
    out_d = nc.dram_tensor("out", [BLOC, C2, N], FP32, kind="ExternalOutput")

    with tile.TileContext(nc) as tc:
        emit_body(nc, tc, dict(
            x=x_d, w1t=w1t_d, w2t=w2t_d, w3t=w3t_d, prew=prew_d,
            protot=protot_d, ctxwt=ctxwt_d, edgewt=edgewt_d, nodewt=nodewt_d,
            b1=b1_d, b2=b2_d, b3=b3_d, eb=eb_d, nb=nb_d, out=out_d))
    return nc


def emit_body(nc, tc, d):
    from contextlib import ExitStack
    ctx = ExitStack()
    with ctx:
        singles = ctx.enter_context(tc.tile_pool(name="singles", bufs=1))
        xs_pool = ctx.enter_context(tc.tile_pool(name="xs", bufs=2))
        tok_pool = ctx.enter_context(tc.tile_pool(name="tok", bufs=2))
        y2_pool = ctx.enter_context(tc.tile_pool(name="y2", bufs=2))
        l2_pool = ctx.enter_context(tc.tile_pool(name="l2", bufs=2))
        sm_pool = ctx.enter_context(tc.tile_pool(name="sm", bufs=1))
        small = ctx.enter_context(tc.tile_pool(name="small", bufs=2))
        stage = ctx.enter_context(tc.tile_pool(name="stage", bufs=3))
        psum = ctx.enter_context(tc.tile_pool(name="psum", bufs=2, space="PSUM"))

        # ------- loads: x + first-needed weights on HWDGE (sync), the rest
        # ------- on the SWDGE (gpsimd) queue so they don't delay the x path
        def ld_w(name, dram, kt, mcols, eng):
            t = singles.tile([128, kt, mcols], BF, tag=name)
            eng.dma_start(out=t, in_=dram[:].rearrange("(t p) m -> p t m", p=128))
            return t

        def ld_b(name, dram, eng):
            t = singles.tile([128, 2], FP32, tag=name)
            eng.dma_start(out=t, in_=dram[:].rearrange("(t p) -> p t", p=128))
            return t

        w1t = ld_w("w1t", d["w1t"], 2, CH, nc.sync)
        b1s = ld_b("b1", d["b1"], nc.sync)

        xs = [xs_pool.tile([128, 2, N], BF, tag="xs", name="xs") for _ in range(BLOC)]
        for b in range(BLOC):
            xr = d["x"][b].rearrange("(t p) n -> p t n", p=128)
            for nch in range(N // NCH):
                nc.sync.dma_start(out=xs[b][:, :, nch * NCH:(nch + 1) * NCH],
                                  in_=xr[:, :, nch * NCH:(nch + 1) * NCH])

        w2t = ld_w("w2t", d["w2t"], 2, CH, nc.gpsimd)
        w3t = ld_w("w3t", d["w3t"], 4, C2, nc.gpsimd)
        prew = ld_w("prew", d["prew"], 2, CH, nc.gpsimd)
        edgewt = ld_w("edgewt", d["edgewt"], 2, CH, nc.gpsimd)
        nodewt = ld_w("nodewt", d["nodewt"], 2, CH, nc.gpsimd)
        ctxwt = ld_w("ctxwt", d["ctxwt"], 4, E * CH, nc.gpsimd)
        protot = ld_w("protot", d["protot"], 2, E, nc.gpsimd)
        b2s, b3s = ld_b("b2", d["b2"], nc.gpsimd), ld_b("b3", d["b3"], nc.gpsimd)
        ebs, nbs = ld_b("eb", d["eb"], nc.gpsimd), ld_b("nb", d["nb"], nc.gpsimd)

        tokens = [tok_pool.tile([128, 2, N], BF, tag="tok", name="tok") for _ in range(BLOC)]
        y2 = [y2_pool.tile([128, 2, N], BF, tag="y2", name="y2") for _ in range(BLOC)]
        tok_sums = [small.tile([128, 2, N // NCH], FP32, tag="tsum", name="tsum") for _ in range(BLOC)]
        # ctx^T: [128, kt(4), b]  kt 0-1 avg halves, kt 2-3 max halves (bf16)
        ctxT = small.tile([128, 4, BLOC], BF, tag="ctxT", name="ctxT")

        # ---------------- Phase Y: y1 (tokens) then y2, both batches ------
        def conv_bn_silu(b, wt, bias_s, out_tile, accum, chunk_hook=None):
            for m in range(2):
                for nch in range(N // NCH):
                    ps = psum.tile([128, NCH], FP32, tag="big", name="big")
                    for kt in range(2):
                        for ns in range(NCH // NSUB):
                            nc.tensor.matmul(
                                ps[:, ns * NSUB:(ns + 1) * NSUB],
                                wt[:, kt, m * 128:(m + 1) * 128],
                                xs[b][:, kt, nch * NCH + ns * NSUB: nch * NCH + (ns + 1) * NSUB],
                                start=(kt == 0), stop=(kt == 1))
                    acc = tok_sums[b][:, m, nch:nch + 1] if accum else None
                    nc.scalar.activation(
                        out_tile[:, m, nch * NCH:(nch + 1) * NCH], ps, AF.Silu,
                        bias=bias_s[:, m:m + 1], accum_out=acc)
                    if chunk_hook is not None:
                        chunk_hook(m, nch)

        # y1 for both batches; each SiLU chunk immediately feeds its slice of
        # the L2 transpose (sync HWDGE) and a partial running max, so both are
        # fully pipelined behind the remaining y1/y2/logits matmuls.
        tl2 = [[l2_pool.tile([128, N // 128, 128], BF, tag="l2", name="l2") for _ in range(2)]
               for _ in range(BLOC)]
        TCH = NCH // 128  # transposed t-tiles per chunk
        maxp = [small.tile([128, 2, N // NCH], FP32, tag="maxp", name="maxp")
                for _ in range(BLOC)]
        for b in range(BLOC):
            def hook(m, nch, b=b):
                nc.sync.dma_start(
                    out=tl2[b][m][:, nch * TCH:(nch + 1) * TCH, :],
                    in_=tokens[b][:, m, nch * NCH:(nch + 1) * NCH], transpose=True)
                nc.vector.reduce_max(maxp[b][:, m, nch:nch + 1],
                                     tokens[b][:, m, nch * NCH:(nch + 1) * NCH], AX.X)
            conv_bn_silu(b, w1t, b1s, tokens[b], accum=True, chunk_hook=hook)
            avg_raw = small.tile([128, 2], FP32, tag="avgr", name="avgr")
            nc.vector.reduce_sum(avg_raw, tok_sums[b], AX.X)
            nc.vector.tensor_scalar_mul(ctxT[:, 0:2, b], avg_raw, 1.0 / N)
            for m in range(2):
                nc.vector.reduce_max(ctxT[:, 2 + m, b:b + 1], maxp[b][:, m, :], AX.X)

        # ---------------- ctx -> offsets -> protosb^T -> Q^T --------------
        # offsets psum [2, E*CH] in 1024-col chunks
        offp = sm_pool.tile([16, E * CH], BF, tag="offp", name="offp")
        for nch in range((E * CH) // 1024):
            ps = psum.tile([BLOC, 1024], FP32, tag="big", name="big")
            for kt in range(4):
                for ns in range(2):
                    nc.tensor.matmul(
                        ps[:, ns * NSUB:(ns + 1) * NSUB],
                        ctxT[:, kt, :],
                        ctxwt[:, kt, nch * 1024 + ns * NSUB: nch * 1024 + (ns + 1) * NSUB],
                        start=(kt == 0), stop=(kt == 3))
            nc.vector.tensor_copy(offp[0:BLOC, nch * 1024:(nch + 1) * 1024], ps)
        offT = small.tile([128, 16, 16], BF, tag="offT", name="offT")
        nc.sync.dma_start(out=offT, in_=offp, transpose=True)

        # protosb^T and Q^T per batch
        qT = [small.tile([128, 2, E], BF, tag="qT", name="qT") for _ in range(BLOC)]
        for b in range(BLOC):
            prT = small.tile([128, 2, E], BF, tag="prT", name="prT")
            for h in range(2):
                nc.vector.tensor_add(prT[:, h, :], protot[:, h, :],
                                     offT[:, h:16:2, b])
            for m in range(2):
                ps = psum.tile([128, E], FP32, tag="big", name="big")
                for kt in range(2):
                    nc.tensor.matmul(ps, prew[:, kt, m * 128:(m + 1) * 128],
                                     prT[:, kt, :], start=(kt == 0), stop=(kt == 1))
                nc.vector.tensor_copy(qT[b][:, m, :], ps)

        # ---------------- logits -> softmax (batches at partitions 0/32) --
        PP = 64  # padded partition count; batch b occupies rows [32b, 32b+8)
        lgs = sm_pool.tile([PP, N], FP32, tag="lgs", name="lgs")
        mxlp = small.tile([PP, 2 * (N // 1024)], FP32, tag="mxlp", name="mxlp")
        for b in range(BLOC):
            for nch in range(N // 1024):
                ps = psum.tile([E, 1024], FP32, tag="big", name="big")
                for kt in range(2):
                    for ns in range(2):
                        nc.tensor.matmul(
                            ps[:, ns * NSUB:(ns + 1) * NSUB],
                            qT[b][:, kt, :],
                            tokens[b][:, kt, nch * 1024 + ns * NSUB: nch * 1024 + (ns + 1) * NSUB],
                            start=(kt == 0), stop=(kt == 1))
                nc.vector.tensor_copy(lgs[b * 32:b * 32 + E, nch * 1024:(nch + 1) * 1024], ps)
                nc.vector.reduce_max(
                    mxlp[b * 32:b * 32 + E, b * (N // 1024) + nch: b * (N // 1024) + nch + 1],
                    lgs[b * 32:b * 32 + E, nch * 1024:(nch + 1) * 1024], AX.X)

        # y2 emitted after logits: PE fills the softmax/transpose stall with it
        for b in range(BLOC):
            conv_bn_silu(b, w2t, b2s, y2[b], accum=False)

        mxl = small.tile([PP, 1], FP32, tag="mxl", name="mxl")
        nc.vector.reduce_max(mxl[0:E, :], mxlp[0:E, 0:N // 1024], AX.X)
        nc.vector.reduce_max(mxl[32:32 + E, :], mxlp[32:32 + E, N // 1024:], AX.X)
        ebias = small.tile([PP, 1], FP32, tag="ebias", name="ebias")
        nc.vector.tensor_scalar_mul(ebias, mxl, -LSCALE)
        Pn = sm_pool.tile([PP, N], BF, tag="Pn", name="Pn")
        Zh = small.tile([PP, 2], FP32, tag="Zh", name="Zh")
        PT = small.tile([128, N // 128, PP], BF, tag="PT", name="PT")
        Pn1 = sm_pool.tile([E, N], BF, tag="Pn1", name="Pn1")
        HN = N // 2
        for h in range(2):
            sl = slice(h * HN, (h + 1) * HN)
            nc.scalar.activation(Pn[:, sl], lgs[:, sl], AF.Exp, bias=ebias,
                                 scale=LSCALE, accum_out=Zh[:, h:h + 1])
            nc.sync.dma_start(out=PT[:, h * (HN // 128):(h + 1) * (HN // 128), :],
                              in_=Pn[:, sl], transpose=True)
            nc.vector.tensor_copy(Pn1[:, sl], Pn[32:32 + E, sl])
        Z = small.tile([PP, 1], FP32, tag="Z", name="Z")
        nc.vector.reduce_sum(Z, Zh, AX.X)
        rz = small.tile([PP, 1], FP32, tag="rz", name="rz")
        nc.vector.reciprocal(rz, Z)
        Pn_b = [Pn[0:E, :], Pn1]

        # ---------------- He -> edge -> W_He per batch --------------------
        whT = [small.tile([E, CH], BF, tag="whT", name="whT") for _ in range(BLOC)]
        for b in range(BLOC):
            he_ps = psum.tile([E, CH], FP32, tag="big", name="big")
            for h in range(2):
                for t in range(N // 128):
                    nc.tensor.matmul(
                        he_ps[:, h * 128:(h + 1) * 128],
                        PT[:, t, b * 32:b * 32 + E],
                        tl2[b][h][:, t, :],
                        start=(t == 0), stop=(t == N // 128 - 1))
            hep = small.tile([16, CH], BF, tag="hep", name="hep")
            nc.vector.tensor_scalar_mul(hep[0:E, :], he_ps, rz[b * 32:b * 32 + E, :])
            heT = small.tile([128, 2, 16], BF, tag="heT", name="heT")
            nc.sync.dma_start(out=heT, in_=hep, transpose=True)
            # edge linear + gelu -> He_out^T [c', e]
            heoT = small.tile([128, 2, E], BF, tag="heoT", name="heoT")
            for m in range(2):
                ps = psum.tile([128, E], FP32, tag="big", name="big")
                for kt in range(2):
                    nc.tensor.matmul(ps, edgewt[:, kt, m * 128:(m + 1) * 128],
                                     heT[:, kt, 0:E], start=(kt == 0), stop=(kt == 1))
                nc.scalar.activation(heoT[:, m, :], ps, AF.Gelu, bias=ebs[:, m:m + 1])
            # W_He^T = He_out^T.T @ node_w^T, then fold 1/Z
            wh_ps = psum.tile([E, CH], FP32, tag="big", name="big")
            for kt in range(2):
                nc.tensor.matmul(wh_ps, heoT[:, kt, :], nodewt[:, kt, :],
                                 start=(kt == 0), stop=(kt == 1))
            nc.vector.tensor_scalar_mul(whT[b], wh_ps, rz[b * 32:b * 32 + E, :])

        # ---------------- node linear + gelu + residual -------------------
        m_out = [xs_pool.tile([128, 2, N], BF, tag="xs", name="xs") for _ in range(BLOC)]
        for b in range(BLOC):
            for m in range(2):
                for nch in range(N // NCH):
                    ps = psum.tile([128, NCH], FP32, tag="big", name="big")
                    for ns in range(NCH // NSUB):
                        nc.tensor.matmul(
                            ps[:, ns * NSUB:(ns + 1) * NSUB],
                            whT[b][:, m * 128:(m + 1) * 128],
                            Pn_b[b][:, nch * NCH + ns * NSUB: nch * NCH + (ns + 1) * NSUB],
                            start=True, stop=True)
                    gel = stage.tile([128, NCH], BF, tag="stage", name="stage")
                    nc.scalar.activation(gel, ps, AF.Gelu, bias=nbs[:, m:m + 1])
                    nc.vector.tensor_add(m_out[b][:, m, nch * NCH:(nch + 1) * NCH],
                                         gel, tokens[b][:, m, nch * NCH:(nch + 1) * NCH])

        # ---------------- cv3 + SiLU + store ------------------------------
        for b in range(BLOC):
            for m in range(2):
                for nch in range(N // NCH):
                    ps = psum.tile([128, NCH], FP32, tag="big", name="big")
                    for kt in range(4):
                        rhs_t = m_out[b] if kt < 2 else y2[b]
                        for ns in range(NCH // NSUB):
                            nc.tensor.matmul(
                                ps[:, ns * NSUB:(ns + 1) * NSUB],
                                w3t[:, kt, m * 128:(m + 1) * 128],
                                rhs_t[:, kt % 2, nch * NCH + ns * NSUB: nch * NCH + (ns + 1) * NSUB],
                                start=(kt == 0), stop=(kt == 3))
                    ostg = stage.tile([128, NCH], FP32, tag="stage", name="stage")
                    nc.scalar.activation(ostg, ps, AF.Silu, bias=b3s[:, m:m + 1])
                    nc.sync.dma_start(
                        out=d["out"][b, m * 128:(m + 1) * 128, nch * NCH:(nch + 1) * NCH],
                        in_=ostg)


@functools.cache
def get_nc():
    nc = bacc.Bacc("TRN2", target_bir_lowering=False, debug=False,
                   enable_asserts=False, num_devices=NCORES)
    emit_kernel(nc)
    nc.finalize()
    return nc


def prep_inputs(inputs):
    """Host-side weight folding + dtype casts. Returns per-core input maps."""
    f32 = np.float32

    def fold(w, g, b, m, v):
        s = (g / np.sqrt(v + EPS)).astype(f32)
        return (np.asarray(w, f32) * s[:, None]), (b - m * s).astype(f32)

    W1, b1 = fold(inputs["cv1_w"], inputs["cv1_g"], inputs["cv1_b"], inputs["cv1_m"], inputs["cv1_v"])
    W2, b2 = fold(inputs["cv2_w"], inputs["cv2_g"], inputs["cv2_b"], inputs["cv2_m"], inputs["cv2_v"])
    W3, b3 = fold(inputs["cv3_w"], inputs["cv3_g"], inputs["cv3_b"], inputs["cv3_m"], inputs["cv3_v"])
    proto_eff = np.asarray(inputs["proto"], f32) + np.asarray(inputs["ctx_b"], f32).reshape(E, CH)

    shared = {
        "w1t": np.ascontiguousarray(W1.T).astype(BF16),
        "w2t": np.ascontiguousarray(W2.T).astype(BF16),
        "w3t": np.ascontiguousarray(W3.T).astype(BF16),
        "prew": np.ascontiguousarray(np.asarray(inputs["pre_w"], f32)).astype(BF16),
        "protot": np.ascontiguousarray(proto_eff.T).astype(BF16),
        "ctxwt": np.ascontiguousarray(np.asarray(inputs["ctx_w"], f32).T).astype(BF16),
        "edgewt": np.ascontiguousarray(np.asarray(inputs["edge_w"], f32).T).astype(BF16),
        "nodewt": np.ascontiguousarray(np.asarray(inputs["node_w"], f32).T).astype(BF16),
        "b1": b1, "b2": b2, "b3": b3,
        "eb": np.asarray(inputs["edge_b"], f32),
        "nb": np.asarray(inputs["node_b"], f32),
    }
    x = np.asarray(inputs["x"], f32).reshape(B, C1, N).astype(BF16)
    in_maps = []
    for c in range(NCORES):
        m = dict(shared)
        m["x"] = np.ascontiguousarray(x[c * BLOC:(c + 1) * BLOC])
        in_maps.append(m)
    return in_maps


def run(inputs, trace=False, **kw):
    nc = get_nc()
    in_maps = prep_inputs(inputs)
    res = run_bass_kernel_spmd(nc, in_maps, list(range(NCORES)), trace=trace, **kw)
    outs = [np.asarray(res.results[i]["out"], np.float32) for i in range(NCORES)]
    full = np.concatenate(outs, axis=0).reshape(B, C2, H, W)
    return full, res


def kernel(**inputs):
    out, _ = run(inputs, trace=False)
    return out

